# revision 84
# baseline (speedup 1.0000x reference)
"""Trainium2 Bass kernel: GPT2-style windowed attention (DecisionTransformer).

Full-input contract: kernel(**inputs) -> [B, S, D] float32.

Sharding: batch*heads across 8 cores (core c -> batch c//4, heads 4*(c%4)..+4).
Each core: column-sliced c_attn, full windowed attention for its 4 heads,
row-sliced c_proj producing a partial [S, D] output; host sums partials
(the "all-reduce") and adds c_proj bias + V-bias contribution once.

Layout / schedule choices (measured on HW, ~125us vs 139us baseline):
  - Q/K projection runs in fp8-e4m3 with perf_mode=DoubleRow (2 weights
    per PE cell -> K=256 contraction per pass, 4 passes for K=1024).
    Host packs x and W_qk into [128, t, 2, n] pair layout (virtual
    contraction row = 128*j + p, validated on HW); W_qk is scaled x64
    into fp8 range and rescaled in the bias-add activation. V projection
    and c_proj stay bf16 (V feeds the output directly; fp8 there would
    eat the whole 2e-2 error budget on short-window rows). Measured rel
    err 1.30e-2 (deterministic).
  - scores matmuls use the zero-padded K=128 formulation (the head's 64
    dims + 64 zero rows). A K=64 row-tiled variant (2 heads concurrent
    via tile_position) is genuinely ~2x cheaper per-op on HW but LOWERS
    the PE duty cycle enough that the HAM clock-gate re-throttles to
    K=4/8 mid-wave; the padded form keeps the PE dense and is ~15us
    faster end-to-end. (Don't "optimize" this back without re-measuring
    throttle_active_nc0_time_ns.)
  - V is projected in [seq, head*dim] layout with 64 ones-columns per
    head, so attn@V lands the softmax denominator REPLICATED in PSUM
    rows 64-127 (matmul cost scales with moving cols only): the
    normalize chain is copy+recip+mult with no partition_broadcast hop.
    V-bias folds into the host-side reduce (softmax rows sum to 1).
  - the whole kernel is software-pipelined: projections advance per
    512-col seq chunk, and attention q-quarters (scores+exp for all 4
    heads, kj-major attn@V, normalize, c_proj slice, output DMA) are
    issued as soon as their chunk dependencies are met. ACT does exp +
    the lighter evac half; DVE does masks/rope/recip/normalize; gpsimd
    tensor ops measured ~4x slower than DVE -- don't offload there.
  - rope: rotate_half via partition-swap SBUF DMAs (sign folded into
    the sin table); kp spread DMAs ride sync+gpsimd.
  - DMA rings are packet-size bound (~100GB/s at 1KB packets vs
    ~260GB/s at 4KB): x is sent chunk-major (8KB contiguous per
    partition per chunk), the output is shipped in sb-PAIRS as 4KB
    packets to a block-major layout the host untiles. Only sync +
    scalar HW-DGE rings carry input (gpsimd software DGE is ~5x
    slower); each dma_start costs ~600ns of queue issue time so loads
    are few and big.
  - 10 prewarm matmuls bridge the ~16.5us engine preamble until the
    first input lands; they also hold the HAM clock-gate open. The
    first TWO seq blocks of x ship twice: once inside the big chunk-0
    DMA and once as a small contiguous "x1" tensor that lands ~4us
    earlier, so proj_v(0)/proj_v(1) (and the whole prologue behind
    them) start as soon as the ring delivers 0.5MB instead of 1.5MB.
"""

import sys

import numpy as np

sys.path.insert(0, "/opt/trn_rl_repo")

B, S, D = 2, 2048, 1024
H, HD = 16, 64
WINDOW = 512
ROPE_BASE = 4000.0
NCORES = 8
NH = 4          # heads per core
KT = D // 128   # 8 contraction tiles for c_attn
NB = S // 128   # 16 seq blocks
WB = WINDOW // 128  # 4 -> band spans up to 5 query blocks per key block
WSCALE = 64.0   # fp8 scale applied to W_qk host-side (undone at evac)


def _build_nc(debug_taps=False):
    import concourse.bass as bass
    from concourse import bacc, library_config, mybir
    import concourse.tile as tile

    f32 = mybir.dt.float32
    bf16 = mybir.dt.bfloat16
    f8 = mybir.dt.float8e4
    Exp = mybir.ActivationFunctionType.Exp
    Ident = mybir.ActivationFunctionType.Identity
    DR = mybir.MatmulPerfMode.DoubleRow
    mult = mybir.AluOpType.mult
    ts = bass.ts
    ds = bass.ds

    nc = bacc.Bacc("TRN2")

    # xc is chunk-major [p, sc, kt, n] so each 512-col chunk loads as one
    # DMA with 8KB-contiguous per-partition packets (ring throughput is
    # packet-size bound: ~100GB/s at 1KB vs ~260GB/s at 4KB packets).
    xc_d = nc.dram_tensor("xc", [128, 4 * KT * 512], bf16, kind="ExternalInput")
    # duplicate of x's first 256 cols, contiguous per partition: lands
    # ~4us before the full 1MB chunk 0, so proj_v(0)/proj_v(1) start
    # that much sooner (the prologue is gated by first-data arrival)
    x1_d = nc.dram_tensor("x1", [128, KT * 256], bf16, kind="ExternalInput")
    x8_d = nc.dram_tensor("x8", [128, 4 * 4 * 2 * 512], f8, kind="ExternalInput")
    wqk8_d = nc.dram_tensor("wqk8", [128, 4 * 2 * 512], f8, kind="ExternalInput")
    wv_d = nc.dram_tensor("wv", [128, KT * 256], bf16, kind="ExternalInput")
    bqk_d = nc.dram_tensor("bqk", [128, 4], f32, kind="ExternalInput")
    wp_d = nc.dram_tensor("wp", [NH * HD, D], bf16, kind="ExternalInput")
    cos2_d = nc.dram_tensor("cos2", [128, S], bf16, kind="ExternalInput")
    sin2_d = nc.dram_tensor("sin2", [128, S], bf16, kind="ExternalInput")
    m04_d = nc.dram_tensor("m04", [128, 256], bf16, kind="ExternalInput")
    # output is block-major [p, sb-pair, 2, 1024]: shipped in sb pairs as
    # 4KB-contiguous per-partition packets; host untiles.
    out_d = nc.dram_tensor("out", [128, NB * D], bf16, kind="ExternalOutput")

    HS = S // 4  # q-quarter span: po is 1 PSUM bank
    QB = NB // 4  # 4 q-blocks per quarter

    with tile.TileContext(nc) as tc:
        nc.gpsimd.load_library(library_config.attn)

        with (
            tc.tile_pool(name="persist", bufs=1) as pp,
            tc.tile_pool(name="psj", bufs=2, space="PSUM") as psj_pool,
            tc.tile_pool(name="ps", bufs=2, space="PSUM") as ps_pool,
            tc.tile_pool(name="pso", bufs=2, space="PSUM") as pso_pool,
            tc.tile_pool(name="xw", bufs=1) as xw_pool,
            tc.tile_pool(name="ropetmp", bufs=6) as tmp_pool,
            tc.tile_pool(name="et", bufs=38) as e_pool,
            tc.tile_pool(name="rb", bufs=4) as rb_pool,
            tc.tile_pool(name="yo", bufs=4) as y_pool,
        ):
            # prewarm tile: zeros, matmul'd while input DMAs land
            zb = pp.tile([128, 512], bf16, tag="zb")
            nc.vector.memset(zb[:], 0.0)

            bqk_t = pp.tile([128, 4], f32, tag="bqk")
            nc.sync.dma_start(bqk_t[:], bqk_d[:])
            # m04[:, 0, :] = diag-block mask (kk<=qq); [:, 1, :] = window
            # edge mask (kk>qq) — applied as one two-region strided op
            m04t = pp.tile([128, 2, 128], bf16, tag="m04")
            nc.sync.dma_start(
                m04t[:], m04_d[:].rearrange("p (a b) -> p a b", a=2)
            )

            qk = [
                pp.tile([128, S], bf16, tag=f"qk{c}", name=f"qk{c}") for c in range(4)
            ]
            # per-head K tiles, zero-padded to K=128 (rows 64-127 = 0) so
            # score matmuls are full-partition: keeps the PE array dense
            # (row-tiled K=64 pairs measured FASTER per-op but starved
            # the HAM duty cycle -> cold clock ate the gains)
            kp = [
                pp.tile([128, S], bf16, tag=f"kp{h}", name=f"kp{h}")
                for h in range(NH)
            ]
            for h in range(NH):
                hb = (h % 2) * 64
                nc.vector.memset(kp[h][64 - hb : 128 - hb, :], 0.0)
            # per head 64 v-cols + 64 ones-cols: attn@V then lands the
            # softmax denominator in PSUM rows 64-127 REPLICATED, so the
            # normalize chain needs no partition_broadcast (PE cost is
            # unchanged -- matmul time scales with moving cols only).
            CV = NH * 128
            vbig = pp.tile([128, NB, CV], bf16, tag="vbig")
            outH = pp.tile([128, 2, S], bf16, tag="outH")
            wpt = pp.tile([128, 2, D], bf16, tag="wpt")

            # fp8 pair-packed operands for the DoubleRow QK projection:
            # x8[p, sc, t, j, n] = x[256t + 128j + p, 512sc + n]
            # w8[p, t, j, m]     = 64 * Wqk[256t + 128j + p, m]
            xb1 = xw_pool.tile([128, KT, 256], bf16, tag="xb1")
            x8big = xw_pool.tile([128, 4, 4, 2, 512], f8, tag="x8big")
            wqk8 = xw_pool.tile([128, 4, 2, 512], f8, tag="wqk8")
            wvt = xw_pool.tile([128, KT, 256], bf16, tag="wvt")
            xbig = xw_pool.tile([128, 4, KT, 512], bf16, tag="xbig")
            cos2 = xw_pool.tile([128, S], bf16, tag="cos2")
            sin2 = xw_pool.tile([128, S], bf16, tag="sin2")

            # Input DMAs on the two HW-DGE rings only (sync + scalar);
            # one big contiguous DMA per chunk, most-critical first.
            # gpsimd (slow software DGE) carries no input -- it is kept
            # free for the rope partition swaps.
            xc_r = xc_d[:].rearrange("p (s k n) -> p s k n", s=4, k=KT)
            x8_r = x8_d[:].rearrange("p (s t j n) -> p s t j n", s=4, t=4, j=2)
            nc.sync.dma_start(
                xb1[:], x1_d[:].rearrange("p (k n) -> p k n", n=256)
            )
            nc.sync.dma_start(xbig[:, 0], xc_r[:, 0])
            nc.sync.dma_start(xbig[:, 1], xc_r[:, 1])
            nc.sync.dma_start(cos2[:], cos2_d[:])
            nc.sync.dma_start(sin2[:], sin2_d[:])
            nc.scalar.dma_start(
                wvt[:], wv_d[:].rearrange("p (k n) -> p k n", n=256)
            )
            nc.scalar.dma_start(
                wqk8[:], wqk8_d[:].rearrange("p (t j n) -> p t j n", t=4, j=2)
            )
            nc.scalar.dma_start(x8big[:, 0], x8_r[:, 0])
            nc.scalar.dma_start(x8big[:, 1], x8_r[:, 1])

            def late_loads():
                # chunk 2/3 + c_proj weights: emitted after the sc=0
                # prologue so the sc=0/1 rope + kp-spread DMAs sit AHEAD
                # of this 4.5MB on the ring (they gate the interleaved
                # kj0-3 scores; this bulk isn't needed until wave 0)
                nc.sync.dma_start(xbig[:, 2], xc_r[:, 2])
                nc.sync.dma_start(xbig[:, 3], xc_r[:, 3])
                nc.sync.dma_start(x8big[:, 2], x8_r[:, 2])
                nc.sync.dma_start(x8big[:, 3], x8_r[:, 3])
                for k2 in range(2):
                    nc.sync.dma_start(wpt[:, k2, :], wp_d[ts(k2, 128), :])

            # PE prewarm: keep the tensor engine busy (and the HAM
            # clock-gate open) while the first input DMAs land.
            for w in range(10):
                psw = psj_pool.tile([128, 512], f32, tag="psj", name="psw")
                nc.tensor.matmul(
                    psw[:], zb[:, 0:128], zb[:, 0:512],
                    start=True, stop=True,
                )

            # ---------------- building blocks ----------------
            eTs = [dict() for _ in range(NH)]  # [h][kj] -> masked exp'd scores

            # ones regions are persistent: set once for all seq blocks
            vball = vbig[:].rearrange("p b (h c) -> p b h c", c=128)
            nc.gpsimd.memset(vball[:, :, :, 64:128], 1.0)

            def proj_v(sb):
                vsb = vbig[:, sb, :].rearrange("p (h c) -> p h c", c=128)
                # prologue proj_v borrows the scores pool (idle until the
                # first scores_kj): decouples its PSUM ring from proj_qk's
                # so neither evacuation engine gates the other's recycle
                pool = ps_pool if sb < 8 else psj_pool
                tag = "ps" if sb < 8 else "psj"
                psv = pool.tile([128, 256], f32, tag=tag, name="psv")
                for kt in range(KT):
                    xsb = (
                        xb1[:, kt, ts(sb, 128)]
                        if sb < 2
                        else xbig[:, sb // 4, kt, ts(sb % 4, 128)]
                    )
                    nc.tensor.matmul(
                        psv[:],
                        xsb,
                        wvt[:, kt, :],
                        start=(kt == 0),
                        stop=(kt == KT - 1),
                    )
                nc.vector.tensor_copy(
                    vsb[:, :, 0:64],
                    psv[:].rearrange("p (h c) -> p h c", c=64),
                )

            def proj_qk(c, sc):
                # prologue k-projections (c>=2, sc<2) borrow the attnv
                # PSUM pool (idle until the first attnv at wave 0): the
                # deeper effective ring removes the qk1-waits-qk0 stall
                if sc < 2 and c >= 2:
                    psb = pso_pool.tile([128, 512], f32, tag="pso", name="psb")
                else:
                    psb = psj_pool.tile([128, 512], f32, tag="psj", name="psb")
                for t in range(4):
                    nc.tensor.matmul(
                        psb[:],
                        wqk8[:, t, :, ts(c, 128)],
                        x8big[:, sc, t, :, :],
                        start=(t == 0),
                        stop=(t == 3),
                        perf_mode=DR,
                    )
                # rescale the x64 fp8 weight scale + add bias on ACT
                # (out = psb * 1/64 + bqk; DVE variant measured slower)
                nc.scalar.activation(
                    qk[c][:, ts(sc, 512)], psb[:], Ident,
                    bias=bqk_t[:, c : c + 1], scale=1.0 / WSCALE,
                )
                # rope: rotate_half via partition-swap SBUF DMAs on the
                # sync/gpsimd queues (sign is folded into the sin table)
                qc = qk[c][:, ts(sc, 512)]
                tmp = tmp_pool.tile([128, 512], bf16, tag="ropetmp", name="tmp")
                if sc < 2:
                    # prologue: ACT is idle, lend its DMA queue to rope
                    dma_engs = [nc.sync, nc.gpsimd, nc.scalar, nc.gpsimd]
                else:
                    dma_engs = [nc.sync, nc.gpsimd, nc.sync, nc.gpsimd]
                for g in range(2):
                    b0 = g * 64
                    dma_engs[2 * g].dma_start(
                        tmp[b0 : b0 + 32, :],
                        qk[c][b0 + 32 : b0 + 64, ts(sc, 512)],
                    )
                    dma_engs[2 * g + 1].dma_start(
                        tmp[b0 + 32 : b0 + 64, :],
                        qk[c][b0 : b0 + 32, ts(sc, 512)],
                    )
                nc.vector.tensor_tensor(
                    tmp[:], tmp[:], sin2[:, ts(sc, 512)], op=mult
                )
                nc.vector.tensor_tensor(qc, qc, cos2[:, ts(sc, 512)], op=mult)
                nc.vector.tensor_add(qc, qc, tmp[:])
                if c >= 2:
                    # spread each roped k head into its padded K=128 tile
                    # (partition-aligned with the head's q rows)
                    h0 = 2 * (c - 2)
                    nc.sync.dma_start(
                        kp[h0][0:64, ts(sc, 512)], qk[c][0:64, ts(sc, 512)]
                    )
                    nc.gpsimd.dma_start(
                        kp[h0 + 1][64:128, ts(sc, 512)], qk[c][64:128, ts(sc, 512)]
                    )

            def scores_mm(h, kj, pss, nq, part):
                # one scores matmul sT[k, q]; part 0 = first 512 q-cols,
                # part 1 = the 128-col band tail. lhsT is the head's
                # zero-padded K=128 tile; the moving q carries both heads'
                # rows (the foreign head is nulled by the zero k rows).
                qt = qk[h // 2]
                n1 = min(512, nq * 128)
                n2 = nq * 128 - n1
                lhs_k = kp[h][:, ts(kj, 128)]
                if part == 0:
                    nc.tensor.matmul(
                        pss[:, 0:n1],
                        lhs_k,
                        qt[:, ds(kj * 128, n1)],
                        start=True,
                        stop=True,
                    )
                elif n2:
                    nc.tensor.matmul(
                        pss[:, 512 : 512 + n2],
                        lhs_k,
                        qt[:, ds(kj * 128 + 512, n2)],
                        start=True,
                        stop=True,
                    )

            def exp_mask(h, kj, pss, nq):
                eT = e_pool.tile([128, 640], bf16, tag="et", name="eT")
                nc.scalar.activation(
                    eT[:, 0 : nq * 128], pss[:, 0 : nq * 128], Exp, scale=0.125
                )
                # banded mask: diag block keeps kk<=qq, window edge kk>qq.
                # Both 128-col regions are masked in one strided DVE op
                # (gpsimd is ~4x slower per op AND its queue carries the
                # kp-spread + rope DMAs -- measured regressions twice).
                if nq == WB + 1:
                    e2 = eT[:, 0:640].rearrange("p (x y) -> p x y", y=128)[
                        :, 0:5:4, :
                    ]
                    nc.vector.tensor_tensor(e2, e2, m04t[:], op=mult)
                else:
                    nc.vector.tensor_tensor(
                        eT[:, 0:128], eT[:, 0:128], m04t[:, 0, :], op=mult
                    )
                eTs[h][kj] = eT

            def evac_q(po, h, qtr):
                # normalize by denominators (PSUM rows 64-127, already
                # replicated by the ones-cols) into outH: stage + recip +
                # mult, no partition_broadcast hop.
                # (reciprocal_approx_fast is a custom-DVE op and CANNOT
                # read PSUM directly -- measured garbage on HW.)
                hb = (h % 2) * 64
                rb = rb_pool.tile([64, HS], f32, tag="rb", name="rb")
                if qtr == 2:
                    nc.vector.tensor_copy(rb[:], po[64:128, :])
                else:
                    nc.scalar.copy(rb[:], po[64:128, :])
                nc.vector.reciprocal_approx_fast(rb[:], rb[:])
                nc.vector.tensor_tensor(
                    outH[hb : hb + 64, h // 2, qtr * HS : (qtr + 1) * HS],
                    po[0:64, :],
                    rb[:],
                    op=mult,
                )

            def attnv(h, qtr):
                # kj-major attn@V: each V block loads once and streams its
                # whole q-span (per-element has_written bits handle the
                # staggered accumulation regions)
                kjlo = max(0, 4 * qtr - WB)
                kjhi = 4 * qtr + QB - 1
                po = pso_pool.tile([128, HS], f32, tag="pso", name="po")
                for kj in range(kjlo, kjhi + 1):
                    qlo = max(4 * qtr, kj)
                    qhi = min(4 * qtr + QB - 1, kj + WB)
                    off = (qlo - kj) * 128
                    n = (qhi - qlo + 1) * 128
                    nc.tensor.matmul(
                        po[:, ds((qlo - 4 * qtr) * 128, n)],
                        vbig[:, kj, h * 128 : h * 128 + 128],
                        eTs[h][kj][:, off : off + n],
                        start=(kj == kjlo),
                        stop=(kj == kjhi),
                    )
                evac_q(po, h, qtr)

            ypair = {}

            def cproj(sb):
                # two 1-bank psum groups so c_proj never contends with the
                # scores pool. Output accumulates in sb pairs and ships as
                # one 4KB-per-partition DMA per pair (ring efficiency).
                if sb % 2 == 0:
                    ypair["t"] = y_pool.tile([128, 2, D], bf16, tag="yo", name="yt")
                yt = ypair["t"]
                half = sb % 2
                for ncol in range(2):
                    psp = psj_pool.tile([128, 512], f32, tag="psj", name="psp")
                    for k2 in range(2):
                        nc.tensor.matmul(
                            psp[:],
                            outH[:, k2, ts(sb, 128)],
                            wpt[:, k2, ts(ncol, 512)],
                            start=(k2 == 0),
                            stop=(k2 == 1),
                        )
                    if (sb + ncol) % 2 == 0:
                        nc.scalar.copy(yt[:, half, ts(ncol, 512)], psp[:])
                    else:
                        nc.vector.tensor_copy(yt[:, half, ts(ncol, 512)], psp[:])
                if half == 1:
                    nc.sync.dma_start(
                        out_d[:, ds((sb - 1) * D, 2 * D)],
                        yt[:].rearrange("p a b -> p (a b)"),
                    )

            def scores_pair(kj, pair):
                nq = min(WB + 1, NB - kj)
                h0, h1 = 2 * pair, 2 * pair + 1
                p0 = ps_pool.tile([128, 640], f32, tag="ps", name="pss")
                scores_mm(h0, kj, p0, nq, 0)
                scores_mm(h0, kj, p0, nq, 1)
                p1 = ps_pool.tile([128, 640], f32, tag="ps", name="pss")
                scores_mm(h1, kj, p1, nq, 0)
                scores_mm(h1, kj, p1, nq, 1)
                exp_mask(h0, kj, p0, nq)
                exp_mask(h1, kj, p1, nq)

            def scores_kj(kj):
                # scores+exp for one key block, head-pair interleaved
                scores_pair(kj, 0)
                scores_pair(kj, 1)

            def wave(qtr):
                # Scores run a full wave ahead: this wave consumes quarter
                # qtr's eT tiles (exp'd during wave qtr-1) and produces
                # quarter qtr+1's. Projection chunks lead the wave so the
                # PE stays dense (and the clock-gate warm) while ACT drains
                # the exp backlog; attn@V heads, new score blocks, and the
                # previous quarter's c_proj interleave so no engine queue
                # builds a deep backlog in front of a dependency.
                ch = qtr + 2
                # wave 0: its first attnv unit depends only on the
                # prologue (eTs kj0-3 + V blocks) -- issue it ahead of
                # the projection block so its PSUM slot recycles while
                # the PE streams projections; attnv(1) stays after the
                # block as PE spacing for the chunk-2 rope chain that
                # scores_kj(4) waits on
                pre = 1 if qtr == 0 else 0
                for i in range(pre):
                    attnv(i, qtr)
                if ch < 4:
                    for j in range(4):
                        proj_v(4 * ch + j)
                        proj_qk((0, 2, 1, 3)[j], ch)
                # c_proj of the previous quarter starts one slot late so
                # its first group never waits on that quarter's last
                # normalization chain
                for i in range(4):
                    if qtr >= 1 and i >= 1:
                        cproj(4 * (qtr - 1) + i - 1)
                    if i >= pre:
                        attnv(i, qtr)
                    if qtr < 3:
                        scores_kj(4 * qtr + 4 + i)
                if qtr >= 1:
                    cproj(4 * (qtr - 1) + 3)

            # ---------------- pipelined schedule ----------------
            # proj_v / proj_qk interleave so the DVE (psv) and ACT (psb)
            # evacuations alternate: neither engine gates PSUM recycling.
            # The kj0-3 scores interleave into the sc=1 projections as
            # soon as their pair's q (chunk 1) and k (chunk 0) are roped:
            # this spreads the 16-exp burst into ACT's idle prologue
            # window instead of piling it up right before wave 0.
            for j in range(4):
                proj_v(j)
                proj_qk((0, 2, 1, 3)[j], 0)
            late_loads()
            proj_v(4)
            proj_qk(0, 1)
            proj_v(5)
            proj_qk(2, 1)
            scores_pair(0, 0)
            scores_pair(1, 0)
            proj_v(6)
            proj_qk(1, 1)
            scores_pair(2, 0)
            scores_pair(3, 0)
            proj_v(7)
            proj_qk(3, 1)
            for kj in range(4):
                scores_pair(kj, 1)
            for qtr in range(4):
                wave(qtr)
            for sb in range(12, 16):
                cproj(sb)

    nc.compile()
    return nc


def _host_inputs(hidden, pos, caw, cab, cpw):
    """Build the 8 per-core input maps."""
    inv = 1.0 / (ROPE_BASE ** (np.arange(0, HD, 2, dtype=np.float32) / HD))
    t = np.arange(S, dtype=np.float32)
    freqs = np.outer(t, inv).astype(np.float32)
    emb = np.concatenate([freqs, freqs], axis=1)  # [S, HD]
    cos = np.cos(emb).astype(np.float32)
    sin = np.sin(emb).astype(np.float32)

    import ml_dtypes

    bf = ml_dtypes.bfloat16
    f8 = ml_dtypes.float8_e4m3
    ii = np.arange(128)
    m0 = (ii[:, None] <= ii[None, :]).astype(bf)
    m4 = (ii[:, None] > ii[None, :]).astype(bf)
    m04 = np.ascontiguousarray(np.concatenate([m0, m4], axis=1))

    xTs, x1s, x8s, cos2s, sin2s = [], [], [], [], []
    for b in range(B):
        xT = np.ascontiguousarray(hidden[b].T)  # [D, S] f32
        # chunk-major bf16: xc[p, sc, kt, n] = xT[128kt+p, 512sc+n]
        xc = (
            xT.reshape(KT, 128, 4, 512)
            .transpose(1, 2, 0, 3)
            .reshape(128, 4 * KT * 512)
        )
        xTs.append(np.ascontiguousarray(xc).astype(bf))
        # first 256 cols duplicated, k-tiled contiguous (fast first DMA)
        x1 = (
            xT[:, 0:256].reshape(KT, 128, 256).transpose(1, 0, 2).reshape(128, -1)
        )
        x1s.append(np.ascontiguousarray(x1).astype(bf))
        # pair-packed fp8: x8[p, sc, t, j, n] = xT[256t+128j+p, 512sc+n]
        x8 = (
            xT.reshape(4, 2, 128, 4, 512)
            .transpose(2, 3, 0, 1, 4)
            .reshape(128, 4 * 4 * 2 * 512)
        )
        x8s.append(np.ascontiguousarray(x8).astype(f8))
        cosT = np.ascontiguousarray(cos[pos[b]].T)  # [HD, S]
        sinT = np.ascontiguousarray(sin[pos[b]].T)
        sinS = np.concatenate([-sinT[:32], sinT[32:]], axis=0)
        cos2s.append(np.tile(cosT, (2, 1)).astype(bf))
        sin2s.append(np.tile(sinS, (2, 1)).astype(bf))

    in_maps = []
    for c in range(NCORES):
        b = c // 4
        h0 = NH * (c % 4)
        col = h0 * HD
        w_q = caw[:, col : col + NH * HD]
        w_k = caw[:, D + col : D + col + NH * HD]
        w_v = caw[:, 2 * D + col : 2 * D + col + NH * HD]
        # fp8 pair-packed W_qk (x64 scale): w8[p, t, j, m]=64*Wqk[256t+128j+p, m]
        wqk = np.concatenate([w_q, w_k], axis=1)  # [D, 512]
        wqk8 = (
            (wqk * WSCALE)
            .reshape(4, 2, 128, 512)
            .transpose(2, 0, 1, 3)
            .reshape(128, 4 * 2 * 512)
        )
        wqk8 = np.ascontiguousarray(wqk8).astype(f8)
        # bf16 k-tiled W_v: wv[p, kt, n] = w_v[128kt+p, n]
        wv = (
            w_v.reshape(KT, 128, 256).transpose(1, 0, 2).reshape(128, KT * 256)
        )
        wv = np.ascontiguousarray(wv).astype(bf)
        b_q = cab[col : col + NH * HD]
        b_k = cab[D + col : D + col + NH * HD]
        bqk = np.ascontiguousarray(
            np.concatenate([b_q, b_k]).reshape(4, 128).T
        )  # [128, 4]: partition = col within tile
        wp = np.ascontiguousarray(cpw[col : col + NH * HD, :]).astype(bf)
        in_maps.append(
            {
                "xc": xTs[b],
                "x1": x1s[b],
                "x8": x8s[b],
                "wqk8": wqk8,
                "wv": wv,
                "bqk": bqk,
                "wp": wp,
                "cos2": cos2s[b],
                "sin2": sin2s[b],
                "m04": m04,
            }
        )
    return in_maps


def _assemble(results, cab, cpw, cpb):
    """Host all-reduce of the 4 per-batch partials + biases.

    The V-bias contribution is position-independent after softmax
    (attn rows sum to 1), so it folds into a constant row vector:
    bias_v @ c_proj_w.
    """
    vrow = cab[2 * D :].astype(np.float32) @ cpw.astype(np.float32)
    bias = cpb.astype(np.float32) + vrow
    y = np.empty((B, S, D), dtype=np.float32)
    for b in range(B):
        acc = None
        for c in range(4 * b, 4 * b + 4):
            # untile block-major [p, sb, 1024] -> [sb*128+p, 1024]
            part = (
                results[c]["out"]
                .astype(np.float32)
                .reshape(128, NB, D)
                .transpose(1, 0, 2)
                .reshape(S, D)
            )
            acc = part if acc is None else acc + part
        y[b] = acc + bias[None, :]
    return y


def kernel(**inputs):
    from concourse import bass_utils

    hidden = np.asarray(inputs["hidden_states"], dtype=np.float32)
    pos = np.asarray(inputs["position_ids"]).astype(np.int64)
    caw = np.asarray(inputs["c_attn_w"], dtype=np.float32)
    cab = np.asarray(inputs["c_attn_b"], dtype=np.float32)
    cpw = np.asarray(inputs["c_proj_w"], dtype=np.float32)
    cpb = np.asarray(inputs["c_proj_b"], dtype=np.float32)

    in_maps = _host_inputs(hidden, pos, caw, cab, cpw)
    nc = _build_nc()
    res = bass_utils.run_bass_kernel_spmd(nc, in_maps, list(range(NCORES)))
    return _assemble(res.results, cab, cpw, cpb)


# revision 85
# speedup vs baseline: 1.0440x; 1.0440x over previous
"""Trainium2 Bass kernel: GPT2-style windowed attention (DecisionTransformer).

Full-input contract: kernel(**inputs) -> [B, S, D] float32.

Sharding: batch*heads across 8 cores (core c -> batch c//4, heads 4*(c%4)..+4).
Each core: column-sliced c_attn, full windowed attention for its 4 heads,
row-sliced c_proj producing a partial [S, D] output; host sums partials
(the "all-reduce") and adds c_proj bias + V-bias contribution once.

Layout / schedule choices (measured on HW, ~125us vs 139us baseline):
  - Q/K projection runs in fp8-e4m3 with perf_mode=DoubleRow (2 weights
    per PE cell -> K=256 contraction per pass, 4 passes for K=1024).
    Host packs x and W_qk into [128, t, 2, n] pair layout (virtual
    contraction row = 128*j + p, validated on HW); W_qk is scaled x64
    into fp8 range and rescaled in the bias-add activation. V projection
    and c_proj stay bf16 (V feeds the output directly; fp8 there would
    eat the whole 2e-2 error budget on short-window rows). Measured rel
    err 1.30e-2 (deterministic).
  - scores matmuls use the zero-padded K=128 formulation (the head's 64
    dims + 64 zero rows). A K=64 row-tiled variant (2 heads concurrent
    via tile_position) is genuinely ~2x cheaper per-op on HW but LOWERS
    the PE duty cycle enough that the HAM clock-gate re-throttles to
    K=4/8 mid-wave; the padded form keeps the PE dense and is ~15us
    faster end-to-end. (Don't "optimize" this back without re-measuring
    throttle_active_nc0_time_ns.)
  - V is projected in [seq, head*dim] layout with 64 ones-columns per
    head, so attn@V lands the softmax denominator REPLICATED in PSUM
    rows 64-127 (matmul cost scales with moving cols only): the
    normalize chain is copy+recip+mult with no partition_broadcast hop.
    V-bias folds into the host-side reduce (softmax rows sum to 1).
  - the whole kernel is software-pipelined: projections advance per
    512-col seq chunk, and attention q-quarters (scores+exp for all 4
    heads, kj-major attn@V, normalize, c_proj slice, output DMA) are
    issued as soon as their chunk dependencies are met. ACT does exp +
    the lighter evac half; DVE does masks/rope/recip/normalize; gpsimd
    tensor ops measured ~4x slower than DVE -- don't offload there.
  - rope: rotate_half via partition-swap SBUF DMAs (sign folded into
    the sin table); kp spread DMAs ride sync+gpsimd.
  - DMA rings are packet-size bound (~100GB/s at 1KB packets vs
    ~260GB/s at 4KB): x is sent chunk-major (8KB contiguous per
    partition per chunk), the output is shipped in sb-PAIRS as 4KB
    packets to a block-major layout the host untiles. Only sync +
    scalar HW-DGE rings carry input (gpsimd software DGE is ~5x
    slower); each dma_start costs ~600ns of queue issue time so loads
    are few and big.
  - 10 prewarm matmuls bridge the ~16.5us engine preamble until the
    first input lands; they also hold the HAM clock-gate open. The
    first TWO seq blocks of x ship twice: once inside the big chunk-0
    DMA and once as a small contiguous "x1" tensor that lands ~4us
    earlier, so proj_v(0)/proj_v(1) (and the whole prologue behind
    them) start as soon as the ring delivers 0.5MB instead of 1.5MB.
"""

import sys

import numpy as np

sys.path.insert(0, "/opt/trn_rl_repo")

B, S, D = 2, 2048, 1024
H, HD = 16, 64
WINDOW = 512
ROPE_BASE = 4000.0
NCORES = 8
NH = 4          # heads per core
KT = D // 128   # 8 contraction tiles for c_attn
NB = S // 128   # 16 seq blocks
WB = WINDOW // 128  # 4 -> band spans up to 5 query blocks per key block
WSCALE = 64.0   # fp8 scale applied to W_qk host-side (undone at evac)


def _build_nc(debug_taps=False):
    import concourse.bass as bass
    from concourse import bacc, library_config, mybir
    import concourse.tile as tile

    f32 = mybir.dt.float32
    bf16 = mybir.dt.bfloat16
    f8 = mybir.dt.float8e4
    Exp = mybir.ActivationFunctionType.Exp
    Ident = mybir.ActivationFunctionType.Identity
    DR = mybir.MatmulPerfMode.DoubleRow
    mult = mybir.AluOpType.mult
    ts = bass.ts
    ds = bass.ds

    nc = bacc.Bacc("TRN2")

    # xc is chunk-major [p, sc, kt, n] so each 512-col chunk loads as one
    # DMA with 8KB-contiguous per-partition packets (ring throughput is
    # packet-size bound: ~100GB/s at 1KB vs ~260GB/s at 4KB packets).
    xc_d = nc.dram_tensor("xc", [128, 4 * KT * 512], bf16, kind="ExternalInput")
    # duplicate of x's first 256 cols, contiguous per partition: lands
    # ~4us before the full 1MB chunk 0, so proj_v(0)/proj_v(1) start
    # that much sooner (the prologue is gated by first-data arrival)
    x1_d = nc.dram_tensor("x1", [128, KT * 256], bf16, kind="ExternalInput")
    x8_d = nc.dram_tensor("x8", [128, 4 * 4 * 2 * 512], f8, kind="ExternalInput")
    wqk8_d = nc.dram_tensor("wqk8", [128, 4 * 2 * 512], f8, kind="ExternalInput")
    wv_d = nc.dram_tensor("wv", [128, KT * 256], bf16, kind="ExternalInput")
    bqk_d = nc.dram_tensor("bqk", [128, 4], f32, kind="ExternalInput")
    wp_d = nc.dram_tensor("wp", [NH * HD, D], bf16, kind="ExternalInput")
    cos2_d = nc.dram_tensor("cos2", [128, S], bf16, kind="ExternalInput")
    sin2_d = nc.dram_tensor("sin2", [128, S], bf16, kind="ExternalInput")
    m04_d = nc.dram_tensor("m04", [128, 256], bf16, kind="ExternalInput")
    # output is block-major [p, sb-pair, 2, 1024]: shipped in sb pairs as
    # 4KB-contiguous per-partition packets; host untiles.
    out_d = nc.dram_tensor("out", [128, NB * D], bf16, kind="ExternalOutput")

    HS = S // 4  # q-quarter span: po is 1 PSUM bank
    QB = NB // 4  # 4 q-blocks per quarter

    with tile.TileContext(nc) as tc:
        nc.gpsimd.load_library(library_config.attn)

        with (
            tc.tile_pool(name="persist", bufs=1) as pp,
            tc.tile_pool(name="psj", bufs=2, space="PSUM") as psj_pool,
            tc.tile_pool(name="ps", bufs=2, space="PSUM") as ps_pool,
            tc.tile_pool(name="pso", bufs=2, space="PSUM") as pso_pool,
            tc.tile_pool(name="xw", bufs=1) as xw_pool,
            tc.tile_pool(name="ropetmp", bufs=6) as tmp_pool,
            tc.tile_pool(name="et", bufs=38) as e_pool,
            tc.tile_pool(name="rb", bufs=4) as rb_pool,
            tc.tile_pool(name="yo", bufs=4) as y_pool,
        ):
            # prewarm tile: zeros, matmul'd while input DMAs land
            zb = pp.tile([128, 512], bf16, tag="zb")
            nc.vector.memset(zb[:], 0.0)

            bqk_t = pp.tile([128, 4], f32, tag="bqk")
            nc.sync.dma_start(bqk_t[:], bqk_d[:])
            # m04[:, 0, :] = diag-block mask (kk<=qq); [:, 1, :] = window
            # edge mask (kk>qq) — applied as one two-region strided op
            m04t = pp.tile([128, 2, 128], bf16, tag="m04")
            nc.sync.dma_start(
                m04t[:], m04_d[:].rearrange("p (a b) -> p a b", a=2)
            )

            qk = [
                pp.tile([128, S], bf16, tag=f"qk{c}", name=f"qk{c}") for c in range(4)
            ]
            # per-head K tiles, zero-padded to K=128 (rows 64-127 = 0) so
            # score matmuls are full-partition: keeps the PE array dense
            # (row-tiled K=64 pairs measured FASTER per-op but starved
            # the HAM duty cycle -> cold clock ate the gains)
            kp = [
                pp.tile([128, S], bf16, tag=f"kp{h}", name=f"kp{h}")
                for h in range(NH)
            ]
            for h in range(NH):
                hb = (h % 2) * 64
                nc.vector.memset(kp[h][64 - hb : 128 - hb, :], 0.0)
            # per head 64 v-cols + 64 ones-cols: attn@V then lands the
            # softmax denominator in PSUM rows 64-127 REPLICATED, so the
            # normalize chain needs no partition_broadcast (PE cost is
            # unchanged -- matmul time scales with moving cols only).
            CV = NH * 128
            vbig = pp.tile([128, NB, CV], bf16, tag="vbig")
            outH = pp.tile([128, 2, S], bf16, tag="outH")
            wpt = pp.tile([128, 2, D], bf16, tag="wpt")

            # fp8 pair-packed operands for the DoubleRow QK projection:
            # x8[p, sc, t, j, n] = x[256t + 128j + p, 512sc + n]
            # w8[p, t, j, m]     = 64 * Wqk[256t + 128j + p, m]
            xb1 = xw_pool.tile([128, KT, 256], bf16, tag="xb1")
            x8big = xw_pool.tile([128, 4, 4, 2, 512], f8, tag="x8big")
            wqk8 = xw_pool.tile([128, 4, 2, 512], f8, tag="wqk8")
            wvt = xw_pool.tile([128, KT, 256], bf16, tag="wvt")
            xbig = xw_pool.tile([128, 4, KT, 512], bf16, tag="xbig")
            cos2 = xw_pool.tile([128, S], bf16, tag="cos2")
            sin2 = xw_pool.tile([128, S], bf16, tag="sin2")

            # Input DMAs on the two HW-DGE rings only (sync + scalar);
            # one big contiguous DMA per chunk, most-critical first.
            # gpsimd (slow software DGE) carries no input -- it is kept
            # free for the rope partition swaps.
            xc_r = xc_d[:].rearrange("p (s k n) -> p s k n", s=4, k=KT)
            x8_r = x8_d[:].rearrange("p (s t j n) -> p s t j n", s=4, t=4, j=2)
            nc.sync.dma_start(
                xb1[:], x1_d[:].rearrange("p (k n) -> p k n", n=256)
            )
            nc.sync.dma_start(xbig[:, 0], xc_r[:, 0])
            nc.sync.dma_start(xbig[:, 1], xc_r[:, 1])
            nc.sync.dma_start(cos2[:], cos2_d[:])
            nc.sync.dma_start(sin2[:], sin2_d[:])
            nc.sync.dma_start(xbig[:, 2], xc_r[:, 2])
            nc.sync.dma_start(xbig[:, 3], xc_r[:, 3])
            nc.scalar.dma_start(
                wvt[:], wv_d[:].rearrange("p (k n) -> p k n", n=256)
            )
            nc.scalar.dma_start(
                wqk8[:], wqk8_d[:].rearrange("p (t j n) -> p t j n", t=4, j=2)
            )
            nc.scalar.dma_start(x8big[:, 0], x8_r[:, 0])
            nc.scalar.dma_start(x8big[:, 1], x8_r[:, 1])
            nc.sync.dma_start(x8big[:, 2], x8_r[:, 2])
            nc.sync.dma_start(x8big[:, 3], x8_r[:, 3])
            for k2 in range(2):
                nc.sync.dma_start(wpt[:, k2, :], wp_d[ts(k2, 128), :])

            # PE prewarm: keep the tensor engine busy (and the HAM
            # clock-gate open) while the first input DMAs land.
            for w in range(10):
                psw = psj_pool.tile([128, 512], f32, tag="psj", name="psw")
                nc.tensor.matmul(
                    psw[:], zb[:, 0:128], zb[:, 0:512],
                    start=True, stop=True,
                )

            # ---------------- building blocks ----------------
            eTs = [dict() for _ in range(NH)]  # [h][kj] -> masked exp'd scores

            # ones regions are persistent: set once for all seq blocks
            vball = vbig[:].rearrange("p b (h c) -> p b h c", c=128)
            nc.gpsimd.memset(vball[:, :, :, 64:128], 1.0)

            def proj_v(sb):
                vsb = vbig[:, sb, :].rearrange("p (h c) -> p h c", c=128)
                # prologue proj_v borrows the scores pool (idle until the
                # first scores_kj): decouples its PSUM ring from proj_qk's
                # so neither evacuation engine gates the other's recycle
                pool = ps_pool if sb < 8 else psj_pool
                tag = "ps" if sb < 8 else "psj"
                psv = pool.tile([128, 256], f32, tag=tag, name="psv")
                for kt in range(KT):
                    xsb = (
                        xb1[:, kt, ts(sb, 128)]
                        if sb < 2
                        else xbig[:, sb // 4, kt, ts(sb % 4, 128)]
                    )
                    nc.tensor.matmul(
                        psv[:],
                        xsb,
                        wvt[:, kt, :],
                        start=(kt == 0),
                        stop=(kt == KT - 1),
                    )
                nc.vector.tensor_copy(
                    vsb[:, :, 0:64],
                    psv[:].rearrange("p (h c) -> p h c", c=64),
                )

            def proj_qk(c, sc):
                # prologue k-projections (c>=2, sc<2) borrow the attnv
                # PSUM pool (idle until the first attnv at wave 0): the
                # deeper effective ring removes the qk1-waits-qk0 stall
                if sc < 2 and c >= 2:
                    psb = pso_pool.tile([128, 512], f32, tag="pso", name="psb")
                else:
                    psb = psj_pool.tile([128, 512], f32, tag="psj", name="psb")
                for t in range(4):
                    nc.tensor.matmul(
                        psb[:],
                        wqk8[:, t, :, ts(c, 128)],
                        x8big[:, sc, t, :, :],
                        start=(t == 0),
                        stop=(t == 3),
                        perf_mode=DR,
                    )
                # rescale the x64 fp8 weight scale + add bias on ACT
                # (out = psb * 1/64 + bqk; DVE variant measured slower)
                nc.scalar.activation(
                    qk[c][:, ts(sc, 512)], psb[:], Ident,
                    bias=bqk_t[:, c : c + 1], scale=1.0 / WSCALE,
                )
                # rope: rotate_half via partition-swap SBUF DMAs on the
                # sync/gpsimd queues (sign is folded into the sin table)
                qc = qk[c][:, ts(sc, 512)]
                tmp = tmp_pool.tile([128, 512], bf16, tag="ropetmp", name="tmp")
                if sc < 2:
                    # prologue: ACT is idle, lend its DMA queue to rope
                    dma_engs = [nc.sync, nc.gpsimd, nc.scalar, nc.gpsimd]
                else:
                    dma_engs = [nc.sync, nc.gpsimd, nc.sync, nc.gpsimd]
                for g in range(2):
                    b0 = g * 64
                    dma_engs[2 * g].dma_start(
                        tmp[b0 : b0 + 32, :],
                        qk[c][b0 + 32 : b0 + 64, ts(sc, 512)],
                    )
                    dma_engs[2 * g + 1].dma_start(
                        tmp[b0 + 32 : b0 + 64, :],
                        qk[c][b0 : b0 + 32, ts(sc, 512)],
                    )
                nc.vector.tensor_tensor(
                    tmp[:], tmp[:], sin2[:, ts(sc, 512)], op=mult
                )
                nc.vector.tensor_tensor(qc, qc, cos2[:, ts(sc, 512)], op=mult)
                nc.vector.tensor_add(qc, qc, tmp[:])
                if c >= 2:
                    # spread each roped k head into its padded K=128 tile
                    # (partition-aligned with the head's q rows)
                    h0 = 2 * (c - 2)
                    nc.sync.dma_start(
                        kp[h0][0:64, ts(sc, 512)], qk[c][0:64, ts(sc, 512)]
                    )
                    nc.gpsimd.dma_start(
                        kp[h0 + 1][64:128, ts(sc, 512)], qk[c][64:128, ts(sc, 512)]
                    )

            def scores_mm(h, kj, pss, nq, part):
                # one scores matmul sT[k, q]; part 0 = first 512 q-cols,
                # part 1 = the 128-col band tail. lhsT is the head's
                # zero-padded K=128 tile; the moving q carries both heads'
                # rows (the foreign head is nulled by the zero k rows).
                qt = qk[h // 2]
                n1 = min(512, nq * 128)
                n2 = nq * 128 - n1
                lhs_k = kp[h][:, ts(kj, 128)]
                if part == 0:
                    nc.tensor.matmul(
                        pss[:, 0:n1],
                        lhs_k,
                        qt[:, ds(kj * 128, n1)],
                        start=True,
                        stop=True,
                    )
                elif n2:
                    nc.tensor.matmul(
                        pss[:, 512 : 512 + n2],
                        lhs_k,
                        qt[:, ds(kj * 128 + 512, n2)],
                        start=True,
                        stop=True,
                    )

            def exp_mask(h, kj, pss, nq):
                eT = e_pool.tile([128, 640], bf16, tag="et", name="eT")
                nc.scalar.activation(
                    eT[:, 0 : nq * 128], pss[:, 0 : nq * 128], Exp, scale=0.125
                )
                # banded mask: diag block keeps kk<=qq, window edge kk>qq.
                # Both 128-col regions are masked in one strided DVE op
                # (gpsimd is ~4x slower per op AND its queue carries the
                # kp-spread + rope DMAs -- measured regressions twice).
                if nq == WB + 1:
                    e2 = eT[:, 0:640].rearrange("p (x y) -> p x y", y=128)[
                        :, 0:5:4, :
                    ]
                    nc.vector.tensor_tensor(e2, e2, m04t[:], op=mult)
                else:
                    nc.vector.tensor_tensor(
                        eT[:, 0:128], eT[:, 0:128], m04t[:, 0, :], op=mult
                    )
                eTs[h][kj] = eT

            def evac_q(po, h, qtr):
                # normalize by denominators (PSUM rows 64-127, already
                # replicated by the ones-cols) into outH: stage + recip +
                # mult, no partition_broadcast hop.
                # (reciprocal_approx_fast is a custom-DVE op and CANNOT
                # read PSUM directly -- measured garbage on HW.)
                hb = (h % 2) * 64
                rb = rb_pool.tile([64, HS], f32, tag="rb", name="rb")
                if qtr == 2:
                    nc.vector.tensor_copy(rb[:], po[64:128, :])
                else:
                    nc.scalar.copy(rb[:], po[64:128, :])
                nc.vector.reciprocal_approx_fast(rb[:], rb[:])
                nc.vector.tensor_tensor(
                    outH[hb : hb + 64, h // 2, qtr * HS : (qtr + 1) * HS],
                    po[0:64, :],
                    rb[:],
                    op=mult,
                )

            def attnv(h, qtr):
                # kj-major attn@V: each V block loads once and streams its
                # whole q-span (per-element has_written bits handle the
                # staggered accumulation regions)
                kjlo = max(0, 4 * qtr - WB)
                kjhi = 4 * qtr + QB - 1
                po = pso_pool.tile([128, HS], f32, tag="pso", name="po")
                for kj in range(kjlo, kjhi + 1):
                    qlo = max(4 * qtr, kj)
                    qhi = min(4 * qtr + QB - 1, kj + WB)
                    off = (qlo - kj) * 128
                    n = (qhi - qlo + 1) * 128
                    nc.tensor.matmul(
                        po[:, ds((qlo - 4 * qtr) * 128, n)],
                        vbig[:, kj, h * 128 : h * 128 + 128],
                        eTs[h][kj][:, off : off + n],
                        start=(kj == kjlo),
                        stop=(kj == kjhi),
                    )
                evac_q(po, h, qtr)

            ypair = {}

            def cproj(sb):
                # two 1-bank psum groups so c_proj never contends with the
                # scores pool. Output accumulates in sb pairs and ships as
                # one 4KB-per-partition DMA per pair (ring efficiency).
                if sb % 2 == 0:
                    ypair["t"] = y_pool.tile([128, 2, D], bf16, tag="yo", name="yt")
                yt = ypair["t"]
                half = sb % 2
                for ncol in range(2):
                    psp = psj_pool.tile([128, 512], f32, tag="psj", name="psp")
                    for k2 in range(2):
                        nc.tensor.matmul(
                            psp[:],
                            outH[:, k2, ts(sb, 128)],
                            wpt[:, k2, ts(ncol, 512)],
                            start=(k2 == 0),
                            stop=(k2 == 1),
                        )
                    if (sb + ncol) % 2 == 0:
                        nc.scalar.copy(yt[:, half, ts(ncol, 512)], psp[:])
                    else:
                        nc.vector.tensor_copy(yt[:, half, ts(ncol, 512)], psp[:])
                if half == 1:
                    nc.sync.dma_start(
                        out_d[:, ds((sb - 1) * D, 2 * D)],
                        yt[:].rearrange("p a b -> p (a b)"),
                    )

            def scores_kj(kj):
                # scores+exp for one key block; the head pair's matmuls
                # are row-tiled so they overlap in the PE array. Order
                # h0p0, h1p0, h1p1, h0p1 keeps weight reloads minimal.
                nq = min(WB + 1, NB - kj)
                for pair in range(2):
                    h0, h1 = 2 * pair, 2 * pair + 1
                    p0 = ps_pool.tile([128, 640], f32, tag="ps", name="pss")
                    scores_mm(h0, kj, p0, nq, 0)
                    scores_mm(h0, kj, p0, nq, 1)
                    p1 = ps_pool.tile([128, 640], f32, tag="ps", name="pss")
                    scores_mm(h1, kj, p1, nq, 0)
                    scores_mm(h1, kj, p1, nq, 1)
                    exp_mask(h0, kj, p0, nq)
                    exp_mask(h1, kj, p1, nq)

            def wave(qtr):
                # Scores run a full wave ahead: this wave consumes quarter
                # qtr's eT tiles (exp'd during wave qtr-1) and produces
                # quarter qtr+1's. Projection chunks lead the wave so the
                # PE stays dense (and the clock-gate warm) while ACT drains
                # the exp backlog; attn@V heads, new score blocks, and the
                # previous quarter's c_proj interleave so no engine queue
                # builds a deep backlog in front of a dependency.
                ch = qtr + 2
                # wave 0: its first attnv unit depends only on the
                # prologue (eTs kj0-3 + V blocks) -- issue it ahead of
                # the projection block so its PSUM slot recycles while
                # the PE streams projections; attnv(1) stays after the
                # block as PE spacing for the chunk-2 rope chain that
                # scores_kj(4) waits on
                pre = 1 if qtr == 0 else 0
                for i in range(pre):
                    attnv(i, qtr)
                if ch < 4:
                    for j in range(4):
                        proj_v(4 * ch + j)
                        proj_qk((0, 2, 1, 3)[j], ch)
                # c_proj of the previous quarter starts one slot late so
                # its first group never waits on that quarter's last
                # normalization chain
                for i in range(4):
                    if qtr >= 1 and i >= 1:
                        cproj(4 * (qtr - 1) + i - 1)
                    if i >= pre:
                        attnv(i, qtr)
                    if qtr < 3:
                        scores_kj(4 * qtr + 4 + i)
                if qtr >= 1:
                    cproj(4 * (qtr - 1) + 3)

            # ---------------- pipelined schedule ----------------
            # proj_v / proj_qk interleave so the DVE (psv) and ACT (psb)
            # evacuations alternate: neither engine gates PSUM recycling
            for sc in range(2):
                for j in range(4):
                    proj_v(4 * sc + j)
                    proj_qk((0, 2, 1, 3)[j], sc)
            for kj in range(4):
                scores_kj(kj)
            for qtr in range(4):
                wave(qtr)
            for sb in range(12, 16):
                cproj(sb)

    nc.compile()
    return nc


def _host_inputs(hidden, pos, caw, cab, cpw):
    """Build the 8 per-core input maps."""
    inv = 1.0 / (ROPE_BASE ** (np.arange(0, HD, 2, dtype=np.float32) / HD))
    t = np.arange(S, dtype=np.float32)
    freqs = np.outer(t, inv).astype(np.float32)
    emb = np.concatenate([freqs, freqs], axis=1)  # [S, HD]
    cos = np.cos(emb).astype(np.float32)
    sin = np.sin(emb).astype(np.float32)

    import ml_dtypes

    bf = ml_dtypes.bfloat16
    f8 = ml_dtypes.float8_e4m3
    ii = np.arange(128)
    m0 = (ii[:, None] <= ii[None, :]).astype(bf)
    m4 = (ii[:, None] > ii[None, :]).astype(bf)
    m04 = np.ascontiguousarray(np.concatenate([m0, m4], axis=1))

    xTs, x1s, x8s, cos2s, sin2s = [], [], [], [], []
    for b in range(B):
        xT = np.ascontiguousarray(hidden[b].T)  # [D, S] f32
        # chunk-major bf16: xc[p, sc, kt, n] = xT[128kt+p, 512sc+n]
        xc = (
            xT.reshape(KT, 128, 4, 512)
            .transpose(1, 2, 0, 3)
            .reshape(128, 4 * KT * 512)
        )
        xTs.append(np.ascontiguousarray(xc).astype(bf))
        # first 256 cols duplicated, k-tiled contiguous (fast first DMA)
        x1 = (
            xT[:, 0:256].reshape(KT, 128, 256).transpose(1, 0, 2).reshape(128, -1)
        )
        x1s.append(np.ascontiguousarray(x1).astype(bf))
        # pair-packed fp8: x8[p, sc, t, j, n] = xT[256t+128j+p, 512sc+n]
        x8 = (
            xT.reshape(4, 2, 128, 4, 512)
            .transpose(2, 3, 0, 1, 4)
            .reshape(128, 4 * 4 * 2 * 512)
        )
        x8s.append(np.ascontiguousarray(x8).astype(f8))
        cosT = np.ascontiguousarray(cos[pos[b]].T)  # [HD, S]
        sinT = np.ascontiguousarray(sin[pos[b]].T)
        sinS = np.concatenate([-sinT[:32], sinT[32:]], axis=0)
        cos2s.append(np.tile(cosT, (2, 1)).astype(bf))
        sin2s.append(np.tile(sinS, (2, 1)).astype(bf))

    in_maps = []
    for c in range(NCORES):
        b = c // 4
        h0 = NH * (c % 4)
        col = h0 * HD
        w_q = caw[:, col : col + NH * HD]
        w_k = caw[:, D + col : D + col + NH * HD]
        w_v = caw[:, 2 * D + col : 2 * D + col + NH * HD]
        # fp8 pair-packed W_qk (x64 scale): w8[p, t, j, m]=64*Wqk[256t+128j+p, m]
        wqk = np.concatenate([w_q, w_k], axis=1)  # [D, 512]
        wqk8 = (
            (wqk * WSCALE)
            .reshape(4, 2, 128, 512)
            .transpose(2, 0, 1, 3)
            .reshape(128, 4 * 2 * 512)
        )
        wqk8 = np.ascontiguousarray(wqk8).astype(f8)
        # bf16 k-tiled W_v: wv[p, kt, n] = w_v[128kt+p, n]
        wv = (
            w_v.reshape(KT, 128, 256).transpose(1, 0, 2).reshape(128, KT * 256)
        )
        wv = np.ascontiguousarray(wv).astype(bf)
        b_q = cab[col : col + NH * HD]
        b_k = cab[D + col : D + col + NH * HD]
        bqk = np.ascontiguousarray(
            np.concatenate([b_q, b_k]).reshape(4, 128).T
        )  # [128, 4]: partition = col within tile
        wp = np.ascontiguousarray(cpw[col : col + NH * HD, :]).astype(bf)
        in_maps.append(
            {
                "xc": xTs[b],
                "x1": x1s[b],
                "x8": x8s[b],
                "wqk8": wqk8,
                "wv": wv,
                "bqk": bqk,
                "wp": wp,
                "cos2": cos2s[b],
                "sin2": sin2s[b],
                "m04": m04,
            }
        )
    return in_maps


def _assemble(results, cab, cpw, cpb):
    """Host all-reduce of the 4 per-batch partials + biases.

    The V-bias contribution is position-independent after softmax
    (attn rows sum to 1), so it folds into a constant row vector:
    bias_v @ c_proj_w.
    """
    vrow = cab[2 * D :].astype(np.float32) @ cpw.astype(np.float32)
    bias = cpb.astype(np.float32) + vrow
    y = np.empty((B, S, D), dtype=np.float32)
    for b in range(B):
        acc = None
        for c in range(4 * b, 4 * b + 4):
            # untile block-major [p, sb, 1024] -> [sb*128+p, 1024]
            part = (
                results[c]["out"]
                .astype(np.float32)
                .reshape(128, NB, D)
                .transpose(1, 0, 2)
                .reshape(S, D)
            )
            acc = part if acc is None else acc + part
        y[b] = acc + bias[None, :]
    return y


def kernel(**inputs):
    from concourse import bass_utils

    hidden = np.asarray(inputs["hidden_states"], dtype=np.float32)
    pos = np.asarray(inputs["position_ids"]).astype(np.int64)
    caw = np.asarray(inputs["c_attn_w"], dtype=np.float32)
    cab = np.asarray(inputs["c_attn_b"], dtype=np.float32)
    cpw = np.asarray(inputs["c_proj_w"], dtype=np.float32)
    cpb = np.asarray(inputs["c_proj_b"], dtype=np.float32)

    in_maps = _host_inputs(hidden, pos, caw, cab, cpw)
    nc = _build_nc()
    res = bass_utils.run_bass_kernel_spmd(nc, in_maps, list(range(NCORES)))
    return _assemble(res.results, cab, cpw, cpb)
